# revision 14
# baseline (speedup 1.0000x reference)
"""Trainium2 Bass kernel for nn_EnhancedTFNLayer (RBF field projection +
diffusion + sampling + LN/linear epilogue), data-parallel over batch on 8 cores.

Approach: the RBF kernel family exp(-(p-g)^2/(2 sigma^2)) over the uniform
grid has low numerical rank. We build (on host, float64, from the *parameter*
inputs only) an orthonormal basis Q [R, G] for field functions, plus fitted
operators so the whole pipeline becomes small R-dim matmuls on device:

  phi[n, j] = exp(-(p_n - c_j)^2 / (2 s^2))     (anchor features, K=3 matmul + Exp)
  C_raw = phi^T @ emb          [R, D]
  C     = Wq^T @ C_raw         (orthonormal coords; field(g) ~= Q[:,g]^T C)
  4x:   T = tanh(Q^T (C W_int) + b_int);  C' = SL C + DT * (Q @ T)
  sampled = phi @ (MQ @ C)     (fitted linear-interp evaluation)
  out = LN2(LN1(sampled + emb) @ W_out + b_out + LN1(...))
"""
import sys
import hashlib
import numpy as np

for _p in ("/opt/trn_rl_repo", "/root/.axon_site/_ro/trn_rl_repo"):
    if _p not in sys.path:
        sys.path.insert(0, _p)

import concourse.bass as bass
import concourse.bacc as bacc
import concourse.tile as tile
from concourse import mybir

F32 = mybir.dt.float32
F32R = mybir.dt.float32r
ACTF = mybir.ActivationFunctionType
ALU = mybir.AluOpType

B, N, G, D = 16, 4096, 1024, 256
NUM_STEPS, DT, EPS = 4, 0.01, 1e-5
R = 128
NT = N // 128            # 32 token tiles per batch
NCHUNK = 8               # phi^T chunks of 512 tokens
BL = 2                   # batches per core
NCORES = 8

_CACHE = {}


# --------------------------------------------------------------------------
# host-side operator fitting (float64; parameter inputs only)
# --------------------------------------------------------------------------
def _host_plan(sigma, alpha, grid, W_int, b_int, W_out, b_out,
               ln1_g, ln1_b, ln2_g, ln2_b):
    rng = np.random.default_rng(0)
    c0 = 1.0 - 2.0 * alpha * DT
    c1 = alpha * DT
    pg = np.linspace(0.0, 1.0, 8193)
    K = np.exp(-((pg[:, None] - grid[None, :]) ** 2) / (2 * sigma * sigma))
    # basis enrichment with synthetic tanh fields (params only, no data)
    nsyn = 384
    sub = rng.choice(len(pg), size=256, replace=False)
    Fsyn = K[sub].T @ rng.standard_normal((256, nsyn))
    Fsyn /= np.abs(Fsyn).max(0, keepdims=True) + 1e-30
    fscale = np.sqrt(N * sigma * np.sqrt(np.pi))          # ~field magnitude per unit emb std
    wnorm = np.linalg.norm(W_int, axis=0)
    wcols = rng.choice(len(wnorm), size=nsyn)
    gains = fscale * wnorm[wcols] * rng.uniform(0.5, 2.0, nsyn)
    Tsyn = np.tanh(Fsyn * gains[None, :])
    Msvd = np.concatenate([K, (Tsyn * 0.1).T], axis=0)
    _, _, Vt = np.linalg.svd(Msvd, full_matrices=False)
    Q = Vt[:R]                                            # [R, G] orthonormal rows
    # anchors
    c = np.linspace(-0.08, 1.08, R)
    s = 2.2 * (c[1] - c[0])
    F = np.exp(-((pg[:, None] - c[None, :]) ** 2) / (2 * s * s))
    Qk = K @ Q.T
    Wq, *_ = np.linalg.lstsq(F, Qk, rcond=1e-8)           # [R, R]
    # diffusion operator in Q coords (exact edge-padded 3-tap applied to Q^T)
    Qt = Q.T
    LQt = c0 * Qt.copy()
    LQt[1:-1] += c1 * (Qt[:-2] + Qt[2:])
    LQt[0] += c1 * (Qt[0] + Qt[1])
    LQt[-1] += c1 * (Qt[-2] + Qt[-1])
    SLQ = Q @ LQt                                         # [R, R]
    # sampling (linear interp of Q columns) fitted over anchors
    u = pg * (G - 1)
    i0 = np.clip(np.floor(u), 0, G - 2).astype(int)
    w = u - i0
    lerpQ = Qt[i0] * (1 - w)[:, None] + Qt[i0 + 1] * w[:, None]
    MQ, *_ = np.linalg.lstsq(F, lerpQ, rcond=1e-5)        # [R, R]

    f32 = lambda x: np.ascontiguousarray(x, dtype=np.float32)
    consts = {
        # phi exponent = p*a1_j + 1*a2_j + p^2*a3 : rhs [3, R] for K=3 matmul
        "anch": f32(np.stack([c / (s * s),
                              -c * c / (2 * s * s),
                              np.full(R, -1.0 / (2 * s * s))])),
        "ones_row": f32(np.ones((1, N))),
        "ident": f32(np.eye(128)),
        "q_sb": f32(Q),                                    # [R, G] lhsT slices for int-mm
        "qt_proj": f32((Qt * DT).reshape(8, 128, R).transpose(1, 0, 2).copy()),  # [128, 8, R]
        "slt": f32(SLQ),                                   # lhsT (K=r, M=r') = SLQ? out=SLQ@C -> lhsT[r,r']=SLQ[r',r] -> SLQ.T
        "wq": f32(Wq),                                     # lhsT for C=Wq^T@Craw
        "mqt": f32(MQ),                                    # lhsT for MC=MQ@C -> lhsT[r,r']=MQ[r',r] -> MQ.T
        "wi": f32(W_int.reshape(2, 128, D).transpose(1, 0, 2).copy()),   # [128, 2, D] rhs tiles
        "wo": f32(W_out.reshape(2, 128, D).transpose(1, 0, 2).copy()),
        "ones_col": f32(np.ones((1, 128))),
        "bint_row": f32(b_int.reshape(1, D)),
        "bout_row": f32(b_out.reshape(1, D)),
        "epsb": f32(np.full((128, 1), EPS)),
        "g1": f32(np.broadcast_to(ln1_g, (128, D))),
        "b1": f32(np.broadcast_to(ln1_b, (128, D))),
        "g2": f32(np.broadcast_to(ln2_g, (128, D))),
        "b2": f32(np.broadcast_to(ln2_b, (128, D))),
    }
    # fix transposed lhsT consts
    consts["slt"] = f32(SLQ.T)
    consts["mqt"] = f32(MQ.T)
    flags = {
        "use_bint": bool(np.any(b_int != 0)),
        "use_bout": bool(np.any(b_out != 0)),
        "ln1_aff": bool(np.any(ln1_g != 1) or np.any(ln1_b != 0)),
        "ln2_aff": bool(np.any(ln2_g != 1) or np.any(ln2_b != 0)),
    }
    return consts, flags


# --------------------------------------------------------------------------
# device module
# --------------------------------------------------------------------------
def _build_module(flags):
    nc = bacc.Bacc(trn_type="TRN2")
    dt_in = {}
    # inputs
    emb_d = nc.dram_tensor("emb", [BL, N, D], F32R, kind="ExternalInput")
    pos_d = nc.dram_tensor("pos", [BL, N, 1], F32, kind="ExternalInput")
    const_specs = {
        "anch": ([3, R], F32), "ones_row": ([1, N], F32),
        "ident": ([128, 128], F32R),
        "q_sb": ([128, G], F32R), "qt_proj": ([128, 8, R], F32R),
        "slt": ([R, R], F32R), "wq": ([R, R], F32R), "mqt": ([R, R], F32R),
        "wi": ([128, 2, D], F32R), "wo": ([128, 2, D], F32R),
        "ones_col": ([1, 128], F32), "bint_row": ([1, D], F32),
        "bout_row": ([1, D], F32),
        "epsb": ([128, 1], F32),
        "g1": ([128, D], F32), "b1": ([128, D], F32),
        "g2": ([128, D], F32), "b2": ([128, D], F32),
    }
    cd = {k: nc.dram_tensor(k, sh, dt, kind="ExternalInput")
          for k, (sh, dt) in const_specs.items()}
    out_d = nc.dram_tensor("out", [BL, N, D], F32, kind="ExternalOutput")
    scratch_d = nc.dram_tensor("scratch", [BL, N], F32, kind="Internal")

    with tile.TileContext(nc) as tc:
        with tc.tile_pool(name="consts", bufs=1) as cp, \
             tc.tile_pool(name="emb", bufs=2) as embp, \
             tc.tile_pool(name="phit", bufs=2) as phitp, \
             tc.tile_pool(name="coef", bufs=2) as coefp, \
             tc.tile_pool(name="pre", bufs=2) as prep, \
             tc.tile_pool(name="work", bufs=3) as wp, \
             tc.tile_pool(name="tiny", bufs=4) as tp, \
             tc.tile_pool(name="ppA", bufs=2, space="PSUM") as ppA, \
             tc.tile_pool(name="ppB", bufs=1, space="PSUM") as ppB:

            # ---- load constants (once) ----
            ct = {}
            for k, (sh, dt) in const_specs.items():
                ct[k] = cp.tile(sh, dt, tag=k, name=f"c_{k}")
                nc.sync.dma_start(ct[k][:], cd[k][tuple(slice(None) for _ in sh)])

            from concourse.tile_rust import add_dep_helper
            for b in range(BL):
                # ============ prologue: emb + pp3 = [p; 1; p^2] [3, N] ============
                emb_sb = embp.tile([128, NT, D], F32R, tag="emb")
                nc.sync.dma_start(emb_sb[:],
                                  emb_d[b].rearrange("(t q) d -> q t d", q=128))
                pp3 = prep.tile([3, N], F32, tag="pp3")
                nc.sync.dma_start(pp3[0:1, :], pos_d[b, :, :].rearrange("n one -> one n"))
                nc.sync.dma_start(pp3[1:2, :], ct["ones_row"][:, :])
                pos_pt = prep.tile([128, NT], F32, tag="pos_pt")
                nc.sync.dma_start(pos_pt[:],
                                  pos_d[b, :, 0].rearrange("(t q) -> q t", q=128))
                psq = prep.tile([128, NT], F32, tag="psq")
                nc.scalar.square(psq[:], pos_pt[:])
                iw = nc.sync.dma_start(scratch_d[b].rearrange("(t q) -> q t", q=128),
                                       psq[:])
                ir = nc.sync.dma_start(pp3[2:3, :],
                                       scratch_d[b].rearrange("(one n) -> one n", one=1))
                add_dep_helper(ir.ins, iw.ins, sync=True, reason="scratch RAW")

                # ============ stage 1: phi^T chunks, phi tiles, C_raw ============
                phiT = [phitp.tile([R, 512], F32R, tag=f"phiT{j}", name=f"phiT_{b}_{j}")
                        for j in range(NCHUNK)]
                pC = ppA.tile([R, D], F32, tag="Cacc")
                for j in range(NCHUNK):
                    pphi = ppB.tile([R, 512], F32, tag="big")
                    nc.tensor.matmul(pphi[:], ct["anch"][:, :],
                                     pp3[:, 512 * j:512 * (j + 1)],
                                     start=True, stop=True)
                    nc.scalar.activation(phiT[j][:], pphi[:], ACTF.Exp)
                    for h in range(4):
                        t = 4 * j + h
                        ptr = ppB.tile([128, 128], F32R, tag="tr", bufs=3)
                        nc.tensor.transpose(ptr[:], phiT[j][:, 128 * h:128 * (h + 1)],
                                            ct["ident"][:, :])
                        phiN = wp.tile([128, R], F32R, tag="phiN")
                        nc.vector.tensor_copy(phiN[:], ptr[:])
                        nc.tensor.matmul(pC[:], phiN[:], emb_sb[:, t, :],
                                         start=(t == 0), stop=(t == NT - 1))
                # C_raw -> orthonormal coords C
                craw = coefp.tile([R, D], F32R, tag="craw")
                nc.scalar.copy(craw[:], pC[:])
                pC2 = ppB.tile([R, D], F32, tag="mm", bufs=2)
                nc.tensor.matmul(pC2[:], ct["wq"][:, :], craw[:], start=True, stop=True)
                C = coefp.tile([R, D], F32R, tag="C")
                nc.scalar.copy(C[:], pC2[:])

                # ============ stage 2: diffusion in coefficient space ============
                for step in range(NUM_STEPS):
                    # C^T [2x128 d, R] via PE transpose
                    Ct = wp.tile([128, 2, R], F32R, tag="Ct")
                    for h in range(2):
                        ptr = ppB.tile([128, 128], F32R, tag="tr", bufs=3)
                        nc.tensor.transpose(ptr[:], C[:, 128 * h:128 * (h + 1)],
                                            ct["ident"][:, :])
                        nc.scalar.copy(Ct[:, h, :], ptr[:])
                    # CW = C @ W_int  [R, D]
                    pCW = ppB.tile([R, D], F32, tag="mm", bufs=2)
                    for h in range(2):
                        nc.tensor.matmul(pCW[:], Ct[:, h, :], ct["wi"][:, h, :],
                                         start=(h == 0), stop=(h == 1))
                    CW = wp.tile([R, D], F32R, tag="CW")
                    nc.scalar.copy(CW[:], pCW[:])
                    # interference per g-tile: psum = Q_g^T? -> [g, e], tanh
                    pCn = ppA.tile([R, D], F32, tag="Cacc")
                    nc.tensor.matmul(pCn[:], ct["slt"][:, :], C[:, :],
                                     start=True, stop=False)
                    for gt in range(8):
                        pint = ppB.tile([128, D], F32, tag="mm", bufs=2)
                        nc.tensor.matmul(pint[:], ct["q_sb"][:, 128 * gt:128 * (gt + 1)],
                                         CW[:], start=True,
                                         stop=not flags["use_bint"])
                        if flags["use_bint"]:
                            nc.tensor.matmul(pint[:], ct["ones_col"][:, :],
                                             ct["bint_row"][:, :],
                                             start=False, stop=True)
                        T = wp.tile([128, D], F32R, tag="Ttile")
                        nc.scalar.activation(T[:], pint[:], ACTF.Tanh)
                        nc.tensor.matmul(pCn[:], ct["qt_proj"][:, gt, :], T[:],
                                         start=False, stop=(gt == 7))
                    C = coefp.tile([R, D], F32R, tag="C")
                    nc.scalar.copy(C[:], pCn[:])

                # MC = MQ @ C
                pMC = ppB.tile([R, D], F32, tag="mm", bufs=2)
                nc.tensor.matmul(pMC[:], ct["mqt"][:, :], C[:], start=True, stop=True)
                MC = coefp.tile([R, D], F32R, tag="MC")
                nc.scalar.copy(MC[:], pMC[:])

                # ============ stage 3: sampling + epilogue per token tile ============
                for t in range(NT):
                    j, h = divmod(t, 4)
                    psamp = ppB.tile([128, D], F32, tag="mm", bufs=2)
                    nc.tensor.matmul(psamp[:], phiT[j][:, 128 * h:128 * (h + 1)],
                                     MC[:], start=True, stop=True)
                    # x = sampled + emb (in psum, in-place) ; accum -> sum(x)
                    s1 = tp.tile([128, 1], F32, tag="s1")
                    nc.vector.scalar_tensor_tensor(
                        psamp[:], psamp[:], 1.0, emb_sb[:, t, :].bitcast(F32),
                        op0=ALU.mult, op1=ALU.add, accum_out=s1[:])
                    # LN1 stats
                    negmu = tp.tile([128, 1], F32, tag="negmu")
                    nc.scalar.mul(negmu[:], s1[:], -1.0 / D)
                    junk = wp.tile([128, D], F32, tag="junk")
                    s2 = tp.tile([128, 1], F32, tag="s2")
                    nc.scalar.activation(junk[:], psamp[:], ACTF.Square,
                                         bias=negmu[:], accum_out=s2[:])
                    rstd = tp.tile([128, 1], F32, tag="rstd")
                    nc.scalar.activation(rstd[:], s2[:], ACTF.Sqrt,
                                         scale=1.0 / D, bias=ct["epsb"][:, :])
                    nc.vector.reciprocal(rstd[:], rstd[:])
                    nbias = tp.tile([128, 1], F32, tag="nbias")
                    nc.vector.scalar_tensor_tensor(nbias[:], negmu[:], 1.0, rstd[:],
                                                   op0=ALU.mult, op1=ALU.mult)
                    enh = wp.tile([128, D], F32R, tag="enh")
                    nc.scalar.activation(enh[:], psamp[:], ACTF.Identity,
                                         bias=nbias[:], scale=rstd[:])
                    if flags["ln1_aff"]:
                        enh2 = wp.tile([128, D], F32R, tag="enh2")
                        nc.vector.tensor_mul(enh2[:], enh[:].bitcast(F32),
                                             ct["g1"][:, :])
                        nc.vector.tensor_add(enh2[:], enh2[:].bitcast(F32),
                                             ct["b1"][:, :])
                        enh = enh2
                    # enh^T via 2 PE transposes packed into one psum tile
                    ptr2 = ppB.tile([128, D], F32R, tag="tr", bufs=3)
                    for h2 in range(2):
                        nc.tensor.transpose(ptr2[:, 128 * h2:128 * (h2 + 1)],
                                            enh[:, 128 * h2:128 * (h2 + 1)],
                                            ct["ident"][:, :])
                    enhT = wp.tile([128, 2, 128], F32R, tag="enhT")
                    (nc.vector.tensor_copy if t % 2 == 0 else nc.scalar.copy)(
                        enhT[:].rearrange("p a b -> p (a b)"), ptr2[:])
                    # out1 = enh @ W_out (+ b_out)
                    pout1 = ppB.tile([128, D], F32, tag="mm", bufs=2)
                    for h2 in range(2):
                        nc.tensor.matmul(pout1[:], enhT[:, h2, :], ct["wo"][:, h2, :],
                                         start=(h2 == 0),
                                         stop=(h2 == 1 and not flags["use_bout"]))
                    if flags["use_bout"]:
                        nc.tensor.matmul(pout1[:], ct["ones_col"][:, :],
                                         ct["bout_row"][:, :], start=False, stop=True)
                    # v = out1 + enh ; accum -> sum
                    v = wp.tile([128, D], F32, tag="v")
                    s1b = tp.tile([128, 1], F32, tag="s1b")
                    nc.vector.scalar_tensor_tensor(
                        v[:], pout1[:], 1.0, enh[:].bitcast(F32),
                        op0=ALU.mult, op1=ALU.add, accum_out=s1b[:])
                    negmu2 = tp.tile([128, 1], F32, tag="negmu2")
                    nc.scalar.mul(negmu2[:], s1b[:], -1.0 / D)
                    junk2 = wp.tile([128, D], F32, tag="junk2")
                    s2b = tp.tile([128, 1], F32, tag="s2b")
                    nc.scalar.activation(junk2[:], v[:], ACTF.Square,
                                         bias=negmu2[:], accum_out=s2b[:])
                    rstd2 = tp.tile([128, 1], F32, tag="rstd2")
                    nc.scalar.activation(rstd2[:], s2b[:], ACTF.Sqrt,
                                         scale=1.0 / D, bias=ct["epsb"][:, :])
                    nc.vector.reciprocal(rstd2[:], rstd2[:])
                    nbias2 = tp.tile([128, 1], F32, tag="nbias2")
                    nc.vector.scalar_tensor_tensor(nbias2[:], negmu2[:], 1.0, rstd2[:],
                                                   op0=ALU.mult, op1=ALU.mult)
                    ot = wp.tile([128, D], F32, tag="ot")
                    nc.vector.tensor_scalar(ot[:], v[:], rstd2[:], nbias2[:],
                                            op0=ALU.mult, op1=ALU.add)
                    if flags["ln2_aff"]:
                        nc.vector.tensor_mul(ot[:], ot[:], ct["g2"][:, :])
                        nc.vector.tensor_add(ot[:], ot[:], ct["b2"][:, :])
                    nc.sync.dma_start(out_d[b, 128 * t:128 * (t + 1), :], ot[:])

    nc.compile()
    return nc


# --------------------------------------------------------------------------
# runner (compiled-callable cache; replicates bass2jax.run_bass_via_pjrt)
# --------------------------------------------------------------------------
def _run(nc, in_maps):
    from concourse.bass_utils import run_bass_kernel_spmd
    return run_bass_kernel_spmd(nc, in_maps, core_ids=list(range(NCORES)))


def kernel(**inputs):
    emb = np.ascontiguousarray(inputs["embeddings"], dtype=np.float32)
    pos = np.ascontiguousarray(inputs["positions"], dtype=np.float32)
    grid = np.asarray(inputs["grid_points"], dtype=np.float64)[0, :, 0]
    params = dict(
        sigma=float(np.asarray(inputs["sigma"])),
        alpha=float(np.asarray(inputs["alpha"])),
        grid=grid,
        W_int=np.asarray(inputs["W_int"], np.float64),
        b_int=np.asarray(inputs["b_int"], np.float64),
        W_out=np.asarray(inputs["W_out"], np.float64),
        b_out=np.asarray(inputs["b_out"], np.float64),
        ln1_g=np.asarray(inputs["ln1_g"], np.float64),
        ln1_b=np.asarray(inputs["ln1_b"], np.float64),
        ln2_g=np.asarray(inputs["ln2_g"], np.float64),
        ln2_b=np.asarray(inputs["ln2_b"], np.float64),
    )
    key = hashlib.sha256(b"".join(np.asarray(v).tobytes() for v in params.values())).hexdigest()
    if key not in _CACHE:
        consts, flags = _host_plan(**params)
        nc = _build_module(flags)
        _CACHE[key] = (nc, consts)
    nc, consts = _CACHE[key]

    in_maps = []
    for c in range(NCORES):
        m = {"emb": emb[BL * c:BL * (c + 1)],
             "pos": pos[BL * c:BL * (c + 1)]}
        m.update(consts)
        in_maps.append(m)
    res = _run(nc, in_maps)
    out = np.concatenate([res.results[c]["out"] for c in range(NCORES)], axis=0)
    return out.astype(np.float32)
